# revision 11
# baseline (speedup 1.0000x reference)
"""GCN (2-layer GraphConv, norm='both') on 8 Trainium2 NeuronCores.

Strategy (graph/data parallel, nodes partitioned across cores):
  - Nodes are partitioned into 8 contiguous shards; each core owns its shard's
    in-edges (edges grouped by dst).  Edges are sorted by dst on the host and
    chunked into 128-dst-node groups; per-chunk tile counts are padded to the
    max across the 8 cores so one SPMD program serves all cores.
  - Dispatch 1 (conv0): each core aggregates its in-edges' 4-wide bf16 source
    payloads (host-laid-out, halo-exchange style) via one-hot indicator
    matmuls in PSUM, folds D_in^-1/2 into the PSUM->SBUF copy, applies W0 + b0
    via chained matmuls, and finishes leaky-relu + next-layer D_out^-1/2 in a
    single scalar-engine activation, emitting its hs shard in bf16.
  - Host concatenates the 8 hs shards (pure layout, no math).
  - Dispatch 2 (conv1): each core gathers hs[src] rows (256B bf16) with SWDGE
    dma_gather calls spread over 4 SWDGE queues (descriptor emission
    parallelizes across Q7 core pairs, ~3.7x emission rate), segment-sums per
    128-dst chunk via bf16 one-hot indicator matmuls in PSUM, folds D_in^-1/2
    into the PSUM copy, applies W1 + b1 via chained matmuls, final copy on the
    scalar engine.
  - Host concatenates the 8 output shards.

All O(E*D) / O(N*D) compute and memory traffic runs on-device; the host does
index manipulation (sort/pad/relabel), normalization constants, and the
4-float-per-edge conv0 payload layout.
"""

import os
from contextlib import ExitStack

import ml_dtypes
import numpy as np

import concourse.bass as bass
import concourse.tile as tile
from concourse import bacc, mybir
from concourse._compat import with_exitstack
from concourse.alu_op_type import AluOpType
from concourse.bass_utils import run_bass_kernel_spmd

F32 = mybir.dt.float32
BF16 = mybir.dt.bfloat16
I16 = mybir.dt.int16
BF = ml_dtypes.bfloat16

NC_CORES = 8
D = 128          # feature dim of both conv layers
NEG_SLOPE = 0.01
NQ = 4           # SWDGE queues for gather descriptor emission

# filled by kernel() for test harnesses to inspect
LAST_EXEC_TIMES_NS: list = []
LAST_RESULTS: list = []


# --------------------------------------------------------------------------
# host-side prep
# --------------------------------------------------------------------------

def _wrap_idx(idx: np.ndarray) -> np.ndarray:
    """dma_gather index layout: position i lives at [i % 16, i // 16] of a
    16-row wrap, replicated 8x (one copy per Q7 core) -> [128, n/16] int16."""
    n = idx.shape[0]
    assert n % 16 == 0
    return np.tile(idx.astype(np.int16).reshape(n // 16, 16).T, (8, 1))


def _prep(src, dst, weight, significance, emb, W0, b0, W1, b1):
    n = weight.shape[0]
    npc = n // NC_CORES                    # nodes per core (owned shard)
    assert npc * NC_CORES == n
    n_chunks = (npc + 127) // 128          # 128-dst-node chunks per core
    half = (n + 1) // 2                    # src-id bucket split for int16 idx
    assert half <= 32767 and n - half <= 32767

    src = np.asarray(src).astype(np.int64)
    dst = np.asarray(dst).astype(np.int64)

    out_deg = np.bincount(src, minlength=n).astype(np.float64)
    in_deg = np.bincount(dst, minlength=n).astype(np.float64)
    od = (1.0 / np.sqrt(np.clip(out_deg, 1.0, None))).astype(np.float32)
    ri = (1.0 / np.sqrt(np.clip(in_deg, 1.0, None))).astype(np.float32)

    # conv0 per-edge source payload (halo-exchange layout):
    #   m_e = od[src] * [w[src], emb[sig[src],0], emb[sig[src],1], 0]
    emb_rows = np.asarray(emb, np.float32)[np.asarray(significance).astype(np.int64)]
    feats = np.concatenate(
        [np.asarray(weight, np.float32)[:, None], emb_rows], axis=1
    ) * od[:, None]                                        # [n, 3]

    order = np.argsort(dst, kind="stable")
    s_src, s_dst = src[order], dst[order]

    core_of = s_dst // npc
    loc = s_dst - core_of * npc
    chunk_of = loc // 128
    e_starts = np.searchsorted(core_of * n_chunks + chunk_of,
                               np.arange(NC_CORES * n_chunks + 1))

    es_all = [[None] * n_chunks for _ in range(NC_CORES)]
    dl_all = [[None] * n_chunks for _ in range(NC_CORES)]
    for c in range(NC_CORES):
        for k in range(n_chunks):
            s0, s1 = e_starts[c * n_chunks + k], e_starts[c * n_chunks + k + 1]
            es_all[c][k] = s_src[s0:s1]
            dl_all[c][k] = (s_dst[s0:s1] - c * npc - k * 128).astype(np.float32)

    # conv0 uses 64-dst chunks (halves the DVE one-hot build work)
    nck0 = (npc + 63) // 64
    chunk_of0 = loc // 64
    e_starts0 = np.searchsorted(core_of * nck0 + chunk_of0,
                                np.arange(NC_CORES * nck0 + 1))
    es0_all = [[None] * nck0 for _ in range(NC_CORES)]
    dl0_all = [[None] * nck0 for _ in range(NC_CORES)]
    for c in range(NC_CORES):
        for k in range(nck0):
            s0, s1 = e_starts0[c * nck0 + k], e_starts0[c * nck0 + k + 1]
            es0_all[c][k] = s_src[s0:s1]
            dl0_all[c][k] = (s_dst[s0:s1] - c * npc - k * 64).astype(np.float32)

    # uniform-across-cores tile counts (SPMD: one program for all cores)
    t0 = np.zeros(nck0, np.int64)
    t1 = np.zeros((n_chunks, 2), np.int64)
    for k in range(nck0):
        ne = max(es0_all[c][k].shape[0] for c in range(NC_CORES))
        t0[k] = max(1, -(-ne // 128))
    for k in range(n_chunks):
        for b in range(2):
            nb = max(int(np.count_nonzero((es_all[c][k] < half) == (b == 0)))
                     for c in range(NC_CORES))
            t1[k, b] = -(-nb // 128)
        if t1[k].sum() == 0:
            t1[k, 0] = 1

    T0 = int(t0.sum())
    T1 = int(t1.sum())

    x0h = np.zeros((NC_CORES, 128, T0 * 4), BF)
    dv0 = np.full((NC_CORES, 128, T0), -1.0, BF)
    idxh = np.zeros((NC_CORES, 128, T1 * 8), np.int16)
    dv1 = np.full((NC_CORES, 128, T1), -1.0, BF)

    off0 = np.concatenate([[0], np.cumsum(t0)])
    off1 = np.concatenate([[0], np.cumsum(t1.reshape(-1))]).reshape(-1)

    for c in range(NC_CORES):
        for k in range(nck0):
            es, dloc = es0_all[c][k], dl0_all[c][k]
            ne = es.shape[0]
            # conv0: all edges of the 64-dst chunk, padded to t0[k]*128
            n0 = int(t0[k]) * 128
            pay = np.zeros((n0, 4), np.float32)
            pay[:ne, :3] = feats[es]
            o = int(off0[k])
            x0h[c, :, o * 4:(o + int(t0[k])) * 4] = (
                pay.reshape(int(t0[k]), 128, 4).transpose(1, 0, 2)
                .reshape(128, int(t0[k]) * 4).astype(BF)
            )
            dvc = np.full(n0, -1.0, np.float32)
            dvc[:ne] = dloc
            dv0[c, :, o:o + int(t0[k])] = dvc.reshape(int(t0[k]), 128).T.astype(BF)

        for k in range(n_chunks):
            es, dloc = es_all[c][k], dl_all[c][k]
            # conv1: bucket by src half, pad idx with 0 (gathered, zeroed by
            # the indicator)
            m0 = es < half
            for b, mask in ((0, m0), (1, ~m0)):
                tb = int(t1[k, b])
                if tb == 0:
                    continue
                nb = tb * 128
                sb = es[mask] - (0 if b == 0 else half)
                db = dloc[mask]
                pidx = np.zeros(nb, np.int64)
                pidx[:sb.shape[0]] = sb
                o1 = int(off1[2 * k + b])
                idxh[c, :, o1 * 8:(o1 + tb) * 8] = _wrap_idx(pidx)
                dvb = np.full(nb, -1.0, np.float32)
                dvb[:db.shape[0]] = db
                dv1[c, :, o1:o1 + tb] = dvb.reshape(tb, 128).T.astype(BF)

    # per-core column-replicated 1/sqrt(in_deg): [128, n_chunks*128] f32
    ri_rows = np.ones((NC_CORES, 128, n_chunks * 128), np.float32)
    od_pc = np.ones((NC_CORES, 128, nck0), np.float32)
    for c in range(NC_CORES):
        rv = np.ones(n_chunks * 128, np.float32)
        rv[:npc] = ri[c * npc:(c + 1) * npc]
        ri_rows[c] = rv[None, :]
        ov = np.ones(nck0 * 64, np.float32)
        ov[:npc] = od[c * npc:(c + 1) * npc]
        od_pc[c, :64, :] = ov.reshape(nck0, 64).T

    t0max = int(t0.max())
    ttmax = int(t1.sum(axis=1).max())
    iota0 = np.tile(np.arange(64, dtype=np.float32)[None, :],
                    (128, t0max)).astype(BF)
    iota1 = np.tile(np.arange(128, dtype=np.float32)[None, :],
                    (128, ttmax)).astype(BF)

    consts = {
        "ones_row": np.ones((1, 128), BF),
        "iota0": iota0,
        "iota1": iota1,
        "b0_row": np.asarray(b0, np.float32)[None, :].astype(BF),
        "b1_row": np.asarray(b1, np.float32)[None, :].astype(BF),
        "W0p": np.concatenate(
            [np.asarray(W0, np.float32), np.zeros((1, D), np.float32)], axis=0
        ).astype(BF),
        "W1": np.asarray(W1, np.float32).astype(BF),
    }
    return dict(
        n=n, npc=npc, n_chunks=n_chunks, nck0=nck0, half=half,
        t0=t0, t1=t1, off0=off0, off1=off1, T0=T0, T1=T1,
        t0max=t0max, ttmax=ttmax, t1max=int(t1.max()),
        od_pc=od_pc, ri_rows=ri_rows, x0h=x0h, dv0=dv0, idxh=idxh, dv1=dv1,
        consts=consts,
    )


# --------------------------------------------------------------------------
# device programs
# --------------------------------------------------------------------------

def _new_nc(nq=1):
    return bacc.Bacc("TRN2", target_bir_lowering=False, debug=False,
                     num_devices=NC_CORES, num_swdge_queues=nq,
                     dynamic_dma_scratch_size=49152)


@with_exitstack
def _conv0_body(ctx: ExitStack, tc, aps, p):
    nc = tc.nc
    nck0, t0, off0, T0 = p["nck0"], p["t0"], p["off0"], p["T0"]
    t0max = p["t0max"]
    cpool = ctx.enter_context(tc.tile_pool(name="consts", bufs=1))
    pool = ctx.enter_context(tc.tile_pool(name="work", bufs=3))
    epool = ctx.enter_context(tc.tile_pool(name="epi", bufs=3))
    ps_a = ctx.enter_context(tc.tile_pool(name="ps_a", bufs=2, space="PSUM"))
    ps_g = ctx.enter_context(tc.tile_pool(name="ps_g", bufs=2, space="PSUM"))

    iota_sb = cpool.tile([128, t0max * 64], BF16)
    nc.sync.dma_start(iota_sb[:], aps["iota0"][:])
    w0_sb = cpool.tile([4, D], BF16)
    nc.sync.dma_start(w0_sb[:], aps["W0p"][:])
    b0r_sb = cpool.tile([1, D], BF16)
    nc.sync.dma_start(b0r_sb[:], aps["b0_row"][:])
    dv0_sb = cpool.tile([128, T0], BF16)
    nc.sync.dma_start(dv0_sb[:], aps["dv0"][:])
    od_sb = cpool.tile([128, nck0], F32)
    nc.sync.dma_start(od_sb[:], aps["od_pc"][:])
    ri_sb = cpool.tile([128, p["n_chunks"] * 128], F32)
    nc.sync.dma_start(ri_sb[:], aps["ri_rows"][:])
    ones1 = cpool.tile([1, 128], BF16)
    nc.sync.dma_start(ones1[:], aps["ones_row"][:])

    x0_sb = cpool.tile([128, T0 * 4], BF16)
    nc.sync.dma_start(x0_sb[:], aps["x0h"][:])
    hs_d = aps["hs"]        # [nck0 * 64, D] bf16 output

    for k in range(nck0):
        tk = int(t0[k])
        o = int(off0[k])
        ind_sb = pool.tile([128, tk * 64], BF16, tag="ind")
        nc.vector.tensor_tensor(
            ind_sb[:].rearrange("p (t j) -> p t j", j=64),
            dv0_sb[:, o:o + tk].unsqueeze(2).broadcast_to([128, tk, 64]),
            iota_sb[:, :tk * 64].rearrange("p (t j) -> p t j", j=64),
            AluOpType.is_equal,
        )
        agg_ps = ps_a.tile([4, 64], F32, tag="agg")
        for t in range(tk):
            nc.tensor.matmul(
                agg_ps[:],
                lhsT=x0_sb[:, (o + t) * 4:(o + t + 1) * 4],
                rhs=ind_sb[:, bass.ts(t, 64)],
                start=(t == 0),
                stop=(t == tk - 1),
            )
        # PSUM->SBUF copy folds the D_in^-1/2 column scale, casts to bf16
        agg_sb = epool.tile([4, 64], BF16, tag="aggsb")
        nc.vector.tensor_tensor(
            agg_sb[:], agg_ps[:], ri_sb[0:4, bass.ts(k, 64)], AluOpType.mult,
        )
        g_ps = ps_g.tile([64, D], F32, tag="g")
        nc.tensor.matmul(g_ps[:], lhsT=agg_sb[:], rhs=w0_sb[:],
                         start=True, stop=False)
        nc.tensor.matmul(g_ps[:], lhsT=ones1[:, :64], rhs=b0r_sb[:],
                         start=False, stop=True)
        # hs = od * leaky_relu(g)  (od > 0 commutes with leaky-relu)
        hs_sb = epool.tile([64, D], BF16, tag="hs")
        nc.scalar.activation(hs_sb[:], g_ps[:], mybir.ActivationFunctionType.Lrelu,
                             scale=od_sb[0:64, k:k + 1], alpha=float(NEG_SLOPE))
        nc.sync.dma_start(hs_d[k * 64:(k + 1) * 64, :], hs_sb[:])


@with_exitstack
def _conv1_body(ctx: ExitStack, tc, aps, p):
    nc = tc.nc
    n_chunks, t1, off1 = p["n_chunks"], p["t1"], p["off1"]
    T1, half, n = p["T1"], p["half"], p["n"]
    ttmax, t1max = p["ttmax"], p["t1max"]
    cpool = ctx.enter_context(tc.tile_pool(name="consts", bufs=1))
    xpool = ctx.enter_context(tc.tile_pool(name="xg", bufs=10))
    ipool = ctx.enter_context(tc.tile_pool(name="ind", bufs=4))
    epool = ctx.enter_context(tc.tile_pool(name="epi", bufs=4))
    ps_a = ctx.enter_context(tc.tile_pool(name="ps_a", bufs=4, space="PSUM"))
    ps_o = ctx.enter_context(tc.tile_pool(name="ps_o", bufs=2, space="PSUM"))

    idx_sb = cpool.tile([128, T1 * 8], I16)
    nc.sync.dma_start(idx_sb[:], aps["idxh"][:])
    iota_sb = cpool.tile([128, ttmax * 128], BF16)
    nc.sync.dma_start(iota_sb[:], aps["iota1"][:])
    w1_sb = cpool.tile([D, D], BF16)
    nc.sync.dma_start(w1_sb[:], aps["W1"][:])
    b1r_sb = cpool.tile([1, D], BF16)
    nc.sync.dma_start(b1r_sb[:], aps["b1_row"][:])
    dv1_sb = cpool.tile([128, T1], BF16)
    nc.sync.dma_start(dv1_sb[:], aps["dv1"][:])
    ri_sb = cpool.tile([128, n_chunks * 128], F32)
    nc.sync.dma_start(ri_sb[:], aps["ri_rows"][:])
    ones1 = cpool.tile([1, 128], BF16)
    nc.sync.dma_start(ones1[:], aps["ones_row"][:])

    hs_d = aps["hs"]        # [n, D] bf16 full pre-scaled features
    out_d = aps["out"]      # [n_chunks * 128, D] f32

    qrr = 0
    for k in range(n_chunks):
        buckets = [(b, int(t1[k, b]), int(off1[2 * k + b]))
                   for b in range(2) if int(t1[k, b]) > 0]
        ttot = sum(tb for _, tb, _ in buckets)

        xgs = []
        for b, tb, o1 in buckets:
            xg = xpool.tile([128, t1max * D], BF16, tag=f"xg{b}")
            src_rows = hs_d[0:half, :] if b == 0 else hs_d[half:n, :]
            nc.gpsimd.dma_gather(
                out_ap=xg[:, :tb * D].rearrange("p (t f) -> p t f", f=D),
                in_ap=src_rows,
                idxs_ap=idx_sb[:, o1 * 8:(o1 + tb) * 8],
                num_idxs=tb * 128,
                num_idxs_reg=tb * 128,
                elem_size=D,
                single_packet=(tb * 8 <= 63),
                queue_num=qrr % NQ,
            )
            qrr += 1
            xgs.append(xg)

        o1_0 = buckets[0][2]
        ind_sb = ipool.tile([128, ttot * 128], BF16, tag="ind")
        nc.vector.tensor_tensor(
            ind_sb[:].rearrange("p (t j) -> p t j", j=128),
            dv1_sb[:, o1_0:o1_0 + ttot].unsqueeze(2).broadcast_to([128, ttot, 128]),
            iota_sb[:, :ttot * 128].rearrange("p (t j) -> p t j", j=128),
            AluOpType.is_equal,
        )
        agg_ps = ps_a.tile([D, 128], F32, tag="agg")
        ti = 0
        for bi, (b, tb, o1) in enumerate(buckets):
            for t in range(tb):
                nc.tensor.matmul(
                    agg_ps[:],
                    lhsT=xgs[bi][:, bass.ts(t, D)],
                    rhs=ind_sb[:, bass.ts(ti, 128)],
                    start=(ti == 0),
                    stop=(ti == ttot - 1),
                )
                ti += 1
        agg_sb = epool.tile([D, 128], BF16, tag="aggsb")
        nc.vector.tensor_tensor(
            agg_sb[:], agg_ps[:], ri_sb[:, bass.ts(k, 128)], AluOpType.mult,
        )
        o_ps = ps_o.tile([128, D], F32, tag="o")
        nc.tensor.matmul(o_ps[:], lhsT=agg_sb[:], rhs=w1_sb[:],
                         start=True, stop=False)
        nc.tensor.matmul(o_ps[:], lhsT=ones1[:], rhs=b1r_sb[:],
                         start=False, stop=True)
        out_sb = epool.tile([128, D], F32, tag="outsb")
        nc.scalar.activation(out_sb[:], o_ps[:],
                             mybir.ActivationFunctionType.Copy)
        nc.sync.dma_start(out_d[k * 128:(k + 1) * 128, :], out_sb[:])


def tensor_specs0(p):
    nck0, T0, t0max = p["nck0"], p["T0"], p["t0max"]
    return {
        "iota0": ((128, t0max * 64), BF16, "ExternalInput"),
        "ones_row": ((1, 128), BF16, "ExternalInput"),
        "b0_row": ((1, D), BF16, "ExternalInput"),
        "W0p": ((4, D), BF16, "ExternalInput"),
        "dv0": ((128, T0), BF16, "ExternalInput"),
        "od_pc": ((128, nck0), F32, "ExternalInput"),
        "ri_rows": ((128, p["n_chunks"] * 128), F32, "ExternalInput"),
        "x0h": ((128, T0 * 4), BF16, "ExternalInput"),
        "hs": ((nck0 * 64, D), BF16, "ExternalOutput"),
    }


def tensor_specs1(p):
    n, n_chunks, T1, ttmax = p["n"], p["n_chunks"], p["T1"], p["ttmax"]
    return {
        "iota1": ((128, ttmax * 128), BF16, "ExternalInput"),
        "ones_row": ((1, 128), BF16, "ExternalInput"),
        "b1_row": ((1, D), BF16, "ExternalInput"),
        "W1": ((D, D), BF16, "ExternalInput"),
        "dv1": ((128, T1), BF16, "ExternalInput"),
        "ri_rows": ((128, n_chunks * 128), F32, "ExternalInput"),
        "idxh": ((128, T1 * 8), I16, "ExternalInput"),
        "hs": ((n, D), BF16, "ExternalInput"),
        "out": ((n_chunks * 128, D), F32, "ExternalOutput"),
    }


def in_maps0(p):
    c = p["consts"]
    return [
        {"iota0": c["iota0"], "ones_row": c["ones_row"],
         "b0_row": c["b0_row"], "W0p": c["W0p"],
         "dv0": p["dv0"][i], "od_pc": p["od_pc"][i], "ri_rows": p["ri_rows"][i],
         "x0h": p["x0h"][i]}
        for i in range(NC_CORES)
    ]


def in_maps1(p, hs_full):
    c = p["consts"]
    return [
        {"iota1": c["iota1"], "ones_row": c["ones_row"],
         "b1_row": c["b1_row"], "W1": c["W1"],
         "dv1": p["dv1"][i], "ri_rows": p["ri_rows"][i], "idxh": p["idxh"][i],
         "hs": hs_full}
        for i in range(NC_CORES)
    ]


def _build(body, tensors, p, nq=1):
    nc = _new_nc(nq)
    aps = {
        name: nc.dram_tensor(name, list(shape), dtype, kind=kind).ap()
        for name, (shape, dtype, kind) in tensors.items()
    }
    with tile.TileContext(nc) as tc:
        body(tc, aps, p)
    nc.compile()
    return nc


# --------------------------------------------------------------------------
# entry point
# --------------------------------------------------------------------------

def kernel(src, dst, weight, significance, emb, W0, b0, W1, b1):
    global LAST_EXEC_TIMES_NS, LAST_RESULTS
    LAST_EXEC_TIMES_NS = []
    LAST_RESULTS = []
    trace = bool(os.environ.get("BASS_TRACE"))

    p = _prep(src, dst, weight, significance, emb, W0, b0, W1, b1)
    n, npc = p["n"], p["npc"]

    nc0 = _build(_conv0_body, tensor_specs0(p), p)
    res0 = run_bass_kernel_spmd(nc0, in_maps0(p), core_ids=list(range(NC_CORES)),
                                trace=trace)
    LAST_RESULTS.append(res0)
    LAST_EXEC_TIMES_NS.append(res0.exec_time_ns)
    hs_full = np.concatenate(
        [np.asarray(res0.results[i]["hs"])[:npc] for i in range(NC_CORES)], axis=0
    )
    assert hs_full.shape == (n, D)

    nc1 = _build(_conv1_body, tensor_specs1(p), p, nq=NQ)
    res1 = run_bass_kernel_spmd(nc1, in_maps1(p, hs_full),
                                core_ids=list(range(NC_CORES)), trace=trace)
    LAST_RESULTS.append(res1)
    LAST_EXEC_TIMES_NS.append(res1.exec_time_ns)

    out = np.concatenate(
        [np.asarray(res1.results[i]["out"])[:npc] for i in range(NC_CORES)], axis=0
    )
    assert out.shape == (n, D)
    return out.astype(np.float32)


# revision 12
# speedup vs baseline: 1.0176x; 1.0176x over previous
"""GCN (2-layer GraphConv, norm='both') on 8 Trainium2 NeuronCores.

Strategy (graph/data parallel, nodes partitioned across cores):
  - Nodes are partitioned into 8 contiguous shards; each core owns its shard's
    in-edges (edges grouped by dst).  Edges are sorted by dst on the host and
    chunked into 128-dst-node groups; per-chunk tile counts are padded to the
    max across the 8 cores so one SPMD program serves all cores.
  - Dispatch 1 (conv0): each core aggregates its in-edges' 4-wide bf16 source
    payloads (host-laid-out, halo-exchange style) via one-hot indicator
    matmuls in PSUM, folds D_in^-1/2 into the PSUM->SBUF copy, applies W0 + b0
    via chained matmuls, and finishes leaky-relu + next-layer D_out^-1/2 in a
    single scalar-engine activation, emitting its hs shard in bf16.
  - Host concatenates the 8 hs shards (pure layout, no math).
  - Dispatch 2 (conv1): each core gathers hs[src] rows (256B bf16) with SWDGE
    dma_gather calls spread over 4 SWDGE queues (descriptor emission
    parallelizes across Q7 core pairs, ~3.7x emission rate), segment-sums per
    128-dst chunk via bf16 one-hot indicator matmuls in PSUM, folds D_in^-1/2
    into the PSUM copy, applies W1 + b1 via chained matmuls, final copy on the
    scalar engine.
  - Host concatenates the 8 output shards.

All O(E*D) / O(N*D) compute and memory traffic runs on-device; the host does
index manipulation (sort/pad/relabel), normalization constants, and the
4-float-per-edge conv0 payload layout.
"""

import os
from contextlib import ExitStack

import ml_dtypes
import numpy as np

import concourse.bass as bass
import concourse.tile as tile
from concourse import bacc, mybir
from concourse._compat import with_exitstack
from concourse.alu_op_type import AluOpType
from concourse.bass_utils import run_bass_kernel_spmd

F32 = mybir.dt.float32
BF16 = mybir.dt.bfloat16
I16 = mybir.dt.int16
BF = ml_dtypes.bfloat16

NC_CORES = 8
D = 128          # feature dim of both conv layers
NEG_SLOPE = 0.01
NQ = 4           # SWDGE queues for gather descriptor emission

# filled by kernel() for test harnesses to inspect
LAST_EXEC_TIMES_NS: list = []
LAST_RESULTS: list = []


# --------------------------------------------------------------------------
# host-side prep
# --------------------------------------------------------------------------

def _wrap_idx(idx: np.ndarray) -> np.ndarray:
    """dma_gather index layout: position i lives at [i % 16, i // 16] of a
    16-row wrap, replicated 8x (one copy per Q7 core) -> [128, n/16] int16."""
    n = idx.shape[0]
    assert n % 16 == 0
    return np.tile(idx.astype(np.int16).reshape(n // 16, 16).T, (8, 1))


def _prep(src, dst, weight, significance, emb, W0, b0, W1, b1):
    n = weight.shape[0]
    npc = n // NC_CORES                    # nodes per core (owned shard)
    assert npc * NC_CORES == n
    n_chunks = (npc + 127) // 128          # 128-dst-node chunks per core
    half = (n + 1) // 2                    # src-id bucket split for int16 idx
    assert half <= 32767 and n - half <= 32767

    src = np.asarray(src).astype(np.int64)
    dst = np.asarray(dst).astype(np.int64)

    out_deg = np.bincount(src, minlength=n).astype(np.float64)
    in_deg = np.bincount(dst, minlength=n).astype(np.float64)
    od = (1.0 / np.sqrt(np.clip(out_deg, 1.0, None))).astype(np.float32)
    ri = (1.0 / np.sqrt(np.clip(in_deg, 1.0, None))).astype(np.float32)

    # conv0 per-edge source payload (halo-exchange layout):
    #   m_e = od[src] * [w[src], emb[sig[src],0], emb[sig[src],1], 0]
    emb_rows = np.asarray(emb, np.float32)[np.asarray(significance).astype(np.int64)]
    feats = np.concatenate(
        [np.asarray(weight, np.float32)[:, None], emb_rows], axis=1
    ) * od[:, None]                                        # [n, 3]

    order = np.argsort(dst, kind="stable")
    s_src, s_dst = src[order], dst[order]

    core_of = s_dst // npc
    loc = s_dst - core_of * npc
    chunk_of = loc // 128
    e_starts = np.searchsorted(core_of * n_chunks + chunk_of,
                               np.arange(NC_CORES * n_chunks + 1))

    es_all = [[None] * n_chunks for _ in range(NC_CORES)]
    dl_all = [[None] * n_chunks for _ in range(NC_CORES)]
    for c in range(NC_CORES):
        for k in range(n_chunks):
            s0, s1 = e_starts[c * n_chunks + k], e_starts[c * n_chunks + k + 1]
            es_all[c][k] = s_src[s0:s1]
            dl_all[c][k] = (s_dst[s0:s1] - c * npc - k * 128).astype(np.float32)

    # conv0 uses 64-dst chunks (halves the DVE one-hot build work)
    nck0 = (npc + 63) // 64
    chunk_of0 = loc // 64
    e_starts0 = np.searchsorted(core_of * nck0 + chunk_of0,
                                np.arange(NC_CORES * nck0 + 1))
    es0_all = [[None] * nck0 for _ in range(NC_CORES)]
    dl0_all = [[None] * nck0 for _ in range(NC_CORES)]
    for c in range(NC_CORES):
        for k in range(nck0):
            s0, s1 = e_starts0[c * nck0 + k], e_starts0[c * nck0 + k + 1]
            es0_all[c][k] = s_src[s0:s1]
            dl0_all[c][k] = (s_dst[s0:s1] - c * npc - k * 64).astype(np.float32)

    # uniform-across-cores tile counts (SPMD: one program for all cores)
    t0 = np.zeros(nck0, np.int64)
    t1 = np.zeros((n_chunks, 2), np.int64)
    for k in range(nck0):
        ne = max(es0_all[c][k].shape[0] for c in range(NC_CORES))
        t0[k] = max(1, -(-ne // 128))
    for k in range(n_chunks):
        for b in range(2):
            nb = max(int(np.count_nonzero((es_all[c][k] < half) == (b == 0)))
                     for c in range(NC_CORES))
            t1[k, b] = -(-nb // 128)
        if t1[k].sum() == 0:
            t1[k, 0] = 1

    T0 = int(t0.sum())
    T1 = int(t1.sum())

    x0h = np.zeros((NC_CORES, 128, T0 * 4), BF)
    dv0 = np.full((NC_CORES, 128, T0), -1.0, BF)
    idxh = np.zeros((NC_CORES, 128, T1 * 8), np.int16)
    dv1 = np.full((NC_CORES, 128, T1), -1.0, BF)

    off0 = np.concatenate([[0], np.cumsum(t0)])
    off1 = np.concatenate([[0], np.cumsum(t1.reshape(-1))]).reshape(-1)

    for c in range(NC_CORES):
        for k in range(nck0):
            es, dloc = es0_all[c][k], dl0_all[c][k]
            ne = es.shape[0]
            # conv0: all edges of the 64-dst chunk, padded to t0[k]*128
            n0 = int(t0[k]) * 128
            pay = np.zeros((n0, 4), np.float32)
            pay[:ne, :3] = feats[es]
            o = int(off0[k])
            x0h[c, :, o * 4:(o + int(t0[k])) * 4] = (
                pay.reshape(int(t0[k]), 128, 4).transpose(1, 0, 2)
                .reshape(128, int(t0[k]) * 4).astype(BF)
            )
            dvc = np.full(n0, -1.0, np.float32)
            dvc[:ne] = dloc
            dv0[c, :, o:o + int(t0[k])] = dvc.reshape(int(t0[k]), 128).T.astype(BF)

        for k in range(n_chunks):
            es, dloc = es_all[c][k], dl_all[c][k]
            # conv1: bucket by src half, pad idx with 0 (gathered, zeroed by
            # the indicator)
            m0 = es < half
            for b, mask in ((0, m0), (1, ~m0)):
                tb = int(t1[k, b])
                if tb == 0:
                    continue
                nb = tb * 128
                sb = es[mask] - (0 if b == 0 else half)
                db = dloc[mask]
                pidx = np.zeros(nb, np.int64)
                pidx[:sb.shape[0]] = sb
                o1 = int(off1[2 * k + b])
                idxh[c, :, o1 * 8:(o1 + tb) * 8] = _wrap_idx(pidx)
                dvb = np.full(nb, -1.0, np.float32)
                dvb[:db.shape[0]] = db
                dv1[c, :, o1:o1 + tb] = dvb.reshape(tb, 128).T.astype(BF)

    # per-core column-replicated 1/sqrt(in_deg): [128, n_chunks*128] f32
    ri_rows = np.ones((NC_CORES, 128, n_chunks * 128), np.float32)
    od_pc = np.ones((NC_CORES, 128, nck0), np.float32)
    for c in range(NC_CORES):
        rv = np.ones(n_chunks * 128, np.float32)
        rv[:npc] = ri[c * npc:(c + 1) * npc]
        ri_rows[c] = rv[None, :]
        ov = np.ones(nck0 * 64, np.float32)
        ov[:npc] = od[c * npc:(c + 1) * npc]
        od_pc[c, :64, :] = ov.reshape(nck0, 64).T

    t0max = int(t0.max())
    ttmax = int(t1.sum(axis=1).max())
    iota0 = np.tile(np.arange(64, dtype=np.float32)[None, :],
                    (128, t0max)).astype(BF)
    iota1 = np.tile(np.arange(128, dtype=np.float32)[None, :],
                    (128, ttmax)).astype(BF)

    consts = {
        "ones_row": np.ones((1, 128), BF),
        "iota0": iota0,
        "iota1": iota1,
        "b0_row": np.asarray(b0, np.float32)[None, :].astype(BF),
        "b1_row": np.asarray(b1, np.float32)[None, :].astype(BF),
        "W0p": np.concatenate(
            [np.asarray(W0, np.float32), np.zeros((1, D), np.float32)], axis=0
        ).astype(BF),
        "W1": np.asarray(W1, np.float32).astype(BF),
    }
    return dict(
        n=n, npc=npc, n_chunks=n_chunks, nck0=nck0, half=half,
        t0=t0, t1=t1, off0=off0, off1=off1, T0=T0, T1=T1,
        t0max=t0max, ttmax=ttmax, t1max=int(t1.max()),
        od_pc=od_pc, ri_rows=ri_rows, x0h=x0h, dv0=dv0, idxh=idxh, dv1=dv1,
        consts=consts,
    )


# --------------------------------------------------------------------------
# device programs
# --------------------------------------------------------------------------

def _new_nc(nq=1):
    return bacc.Bacc("TRN2", target_bir_lowering=False, debug=False,
                     num_devices=NC_CORES, num_swdge_queues=nq)


@with_exitstack
def _conv0_body(ctx: ExitStack, tc, aps, p):
    nc = tc.nc
    nck0, t0, off0, T0 = p["nck0"], p["t0"], p["off0"], p["T0"]
    t0max = p["t0max"]
    cpool = ctx.enter_context(tc.tile_pool(name="consts", bufs=1))
    pool = ctx.enter_context(tc.tile_pool(name="work", bufs=3))
    epool = ctx.enter_context(tc.tile_pool(name="epi", bufs=3))
    ps_a = ctx.enter_context(tc.tile_pool(name="ps_a", bufs=2, space="PSUM"))
    ps_g = ctx.enter_context(tc.tile_pool(name="ps_g", bufs=2, space="PSUM"))

    iota_sb = cpool.tile([128, t0max * 64], BF16)
    nc.sync.dma_start(iota_sb[:], aps["iota0"][:])
    w0_sb = cpool.tile([4, D], BF16)
    nc.sync.dma_start(w0_sb[:], aps["W0p"][:])
    b0r_sb = cpool.tile([1, D], BF16)
    nc.sync.dma_start(b0r_sb[:], aps["b0_row"][:])
    dv0_sb = cpool.tile([128, T0], BF16)
    nc.sync.dma_start(dv0_sb[:], aps["dv0"][:])
    od_sb = cpool.tile([128, nck0], F32)
    nc.sync.dma_start(od_sb[:], aps["od_pc"][:])
    ri_sb = cpool.tile([128, p["n_chunks"] * 128], F32)
    nc.sync.dma_start(ri_sb[:], aps["ri_rows"][:])
    ones1 = cpool.tile([1, 128], BF16)
    nc.sync.dma_start(ones1[:], aps["ones_row"][:])

    x0_sb = cpool.tile([128, T0 * 4], BF16)
    nc.sync.dma_start(x0_sb[:], aps["x0h"][:])
    hs_d = aps["hs"]        # [nck0 * 64, D] bf16 output

    for k in range(nck0):
        tk = int(t0[k])
        o = int(off0[k])
        ind_sb = pool.tile([128, tk * 64], BF16, tag="ind")
        nc.vector.tensor_tensor(
            ind_sb[:].rearrange("p (t j) -> p t j", j=64),
            dv0_sb[:, o:o + tk].unsqueeze(2).broadcast_to([128, tk, 64]),
            iota_sb[:, :tk * 64].rearrange("p (t j) -> p t j", j=64),
            AluOpType.is_equal,
        )
        agg_ps = ps_a.tile([4, 64], F32, tag="agg")
        for t in range(tk):
            nc.tensor.matmul(
                agg_ps[:],
                lhsT=x0_sb[:, (o + t) * 4:(o + t + 1) * 4],
                rhs=ind_sb[:, bass.ts(t, 64)],
                start=(t == 0),
                stop=(t == tk - 1),
            )
        # PSUM->SBUF copy folds the D_in^-1/2 column scale, casts to bf16
        agg_sb = epool.tile([4, 64], BF16, tag="aggsb")
        nc.vector.tensor_tensor(
            agg_sb[:], agg_ps[:], ri_sb[0:4, bass.ts(k, 64)], AluOpType.mult,
        )
        g_ps = ps_g.tile([64, D], F32, tag="g")
        nc.tensor.matmul(g_ps[:], lhsT=agg_sb[:], rhs=w0_sb[:],
                         start=True, stop=False)
        nc.tensor.matmul(g_ps[:], lhsT=ones1[:, :64], rhs=b0r_sb[:],
                         start=False, stop=True)
        # hs = od * leaky_relu(g)  (od > 0 commutes with leaky-relu)
        hs_sb = epool.tile([64, D], BF16, tag="hs")
        nc.scalar.activation(hs_sb[:], g_ps[:], mybir.ActivationFunctionType.Lrelu,
                             scale=od_sb[0:64, k:k + 1], alpha=float(NEG_SLOPE))
        nc.sync.dma_start(hs_d[k * 64:(k + 1) * 64, :], hs_sb[:])


@with_exitstack
def _conv1_body(ctx: ExitStack, tc, aps, p):
    nc = tc.nc
    n_chunks, t1, off1 = p["n_chunks"], p["t1"], p["off1"]
    T1, half, n = p["T1"], p["half"], p["n"]
    ttmax, t1max = p["ttmax"], p["t1max"]
    cpool = ctx.enter_context(tc.tile_pool(name="consts", bufs=1))
    xpool = ctx.enter_context(tc.tile_pool(name="xg", bufs=10))
    ipool = ctx.enter_context(tc.tile_pool(name="ind", bufs=4))
    epool = ctx.enter_context(tc.tile_pool(name="epi", bufs=4))
    ps_a = ctx.enter_context(tc.tile_pool(name="ps_a", bufs=4, space="PSUM"))
    ps_o = ctx.enter_context(tc.tile_pool(name="ps_o", bufs=2, space="PSUM"))

    idx_sb = cpool.tile([128, T1 * 8], I16)
    nc.sync.dma_start(idx_sb[:], aps["idxh"][:])
    iota_sb = cpool.tile([128, ttmax * 128], BF16)
    nc.sync.dma_start(iota_sb[:], aps["iota1"][:])
    w1_sb = cpool.tile([D, D], BF16)
    nc.sync.dma_start(w1_sb[:], aps["W1"][:])
    b1r_sb = cpool.tile([1, D], BF16)
    nc.sync.dma_start(b1r_sb[:], aps["b1_row"][:])
    dv1_sb = cpool.tile([128, T1], BF16)
    nc.sync.dma_start(dv1_sb[:], aps["dv1"][:])
    ri_sb = cpool.tile([128, n_chunks * 128], F32)
    nc.sync.dma_start(ri_sb[:], aps["ri_rows"][:])
    ones1 = cpool.tile([1, 128], BF16)
    nc.sync.dma_start(ones1[:], aps["ones_row"][:])

    hs_d = aps["hs"]        # [n, D] bf16 full pre-scaled features
    out_d = aps["out"]      # [n_chunks * 128, D] f32

    qrr = 0
    for k in range(n_chunks):
        buckets = [(b, int(t1[k, b]), int(off1[2 * k + b]))
                   for b in range(2) if int(t1[k, b]) > 0]
        ttot = sum(tb for _, tb, _ in buckets)

        xgs = []
        for b, tb, o1 in buckets:
            xg = xpool.tile([128, t1max * D], BF16, tag=f"xg{b}")
            src_rows = hs_d[0:half, :] if b == 0 else hs_d[half:n, :]
            nc.gpsimd.dma_gather(
                out_ap=xg[:, :tb * D].rearrange("p (t f) -> p t f", f=D),
                in_ap=src_rows,
                idxs_ap=idx_sb[:, o1 * 8:(o1 + tb) * 8],
                num_idxs=tb * 128,
                num_idxs_reg=tb * 128,
                elem_size=D,
                single_packet=(tb * 8 <= 63),
                queue_num=qrr % NQ,
            )
            qrr += 1
            xgs.append(xg)

        o1_0 = buckets[0][2]
        ind_sb = ipool.tile([128, ttot * 128], BF16, tag="ind")
        nc.vector.tensor_tensor(
            ind_sb[:].rearrange("p (t j) -> p t j", j=128),
            dv1_sb[:, o1_0:o1_0 + ttot].unsqueeze(2).broadcast_to([128, ttot, 128]),
            iota_sb[:, :ttot * 128].rearrange("p (t j) -> p t j", j=128),
            AluOpType.is_equal,
        )
        agg_ps = ps_a.tile([D, 128], F32, tag="agg")
        ti = 0
        for bi, (b, tb, o1) in enumerate(buckets):
            for t in range(tb):
                nc.tensor.matmul(
                    agg_ps[:],
                    lhsT=xgs[bi][:, bass.ts(t, D)],
                    rhs=ind_sb[:, bass.ts(ti, 128)],
                    start=(ti == 0),
                    stop=(ti == ttot - 1),
                )
                ti += 1
        agg_sb = epool.tile([D, 128], BF16, tag="aggsb")
        nc.vector.tensor_tensor(
            agg_sb[:], agg_ps[:], ri_sb[:, bass.ts(k, 128)], AluOpType.mult,
        )
        o_ps = ps_o.tile([128, D], F32, tag="o")
        nc.tensor.matmul(o_ps[:], lhsT=agg_sb[:], rhs=w1_sb[:],
                         start=True, stop=False)
        nc.tensor.matmul(o_ps[:], lhsT=ones1[:], rhs=b1r_sb[:],
                         start=False, stop=True)
        out_sb = epool.tile([128, D], F32, tag="outsb")
        nc.scalar.activation(out_sb[:], o_ps[:],
                             mybir.ActivationFunctionType.Copy)
        nc.sync.dma_start(out_d[k * 128:(k + 1) * 128, :], out_sb[:])


def tensor_specs0(p):
    nck0, T0, t0max = p["nck0"], p["T0"], p["t0max"]
    return {
        "iota0": ((128, t0max * 64), BF16, "ExternalInput"),
        "ones_row": ((1, 128), BF16, "ExternalInput"),
        "b0_row": ((1, D), BF16, "ExternalInput"),
        "W0p": ((4, D), BF16, "ExternalInput"),
        "dv0": ((128, T0), BF16, "ExternalInput"),
        "od_pc": ((128, nck0), F32, "ExternalInput"),
        "ri_rows": ((128, p["n_chunks"] * 128), F32, "ExternalInput"),
        "x0h": ((128, T0 * 4), BF16, "ExternalInput"),
        "hs": ((nck0 * 64, D), BF16, "ExternalOutput"),
    }


def tensor_specs1(p):
    n, n_chunks, T1, ttmax = p["n"], p["n_chunks"], p["T1"], p["ttmax"]
    return {
        "iota1": ((128, ttmax * 128), BF16, "ExternalInput"),
        "ones_row": ((1, 128), BF16, "ExternalInput"),
        "b1_row": ((1, D), BF16, "ExternalInput"),
        "W1": ((D, D), BF16, "ExternalInput"),
        "dv1": ((128, T1), BF16, "ExternalInput"),
        "ri_rows": ((128, n_chunks * 128), F32, "ExternalInput"),
        "idxh": ((128, T1 * 8), I16, "ExternalInput"),
        "hs": ((n, D), BF16, "ExternalInput"),
        "out": ((n_chunks * 128, D), F32, "ExternalOutput"),
    }


def in_maps0(p):
    c = p["consts"]
    return [
        {"iota0": c["iota0"], "ones_row": c["ones_row"],
         "b0_row": c["b0_row"], "W0p": c["W0p"],
         "dv0": p["dv0"][i], "od_pc": p["od_pc"][i], "ri_rows": p["ri_rows"][i],
         "x0h": p["x0h"][i]}
        for i in range(NC_CORES)
    ]


def in_maps1(p, hs_full):
    c = p["consts"]
    return [
        {"iota1": c["iota1"], "ones_row": c["ones_row"],
         "b1_row": c["b1_row"], "W1": c["W1"],
         "dv1": p["dv1"][i], "ri_rows": p["ri_rows"][i], "idxh": p["idxh"][i],
         "hs": hs_full}
        for i in range(NC_CORES)
    ]


def _build(body, tensors, p, nq=1):
    nc = _new_nc(nq)
    aps = {
        name: nc.dram_tensor(name, list(shape), dtype, kind=kind).ap()
        for name, (shape, dtype, kind) in tensors.items()
    }
    with tile.TileContext(nc) as tc:
        body(tc, aps, p)
    nc.compile()
    return nc


# --------------------------------------------------------------------------
# entry point
# --------------------------------------------------------------------------

def kernel(src, dst, weight, significance, emb, W0, b0, W1, b1):
    global LAST_EXEC_TIMES_NS, LAST_RESULTS
    LAST_EXEC_TIMES_NS = []
    LAST_RESULTS = []
    trace = bool(os.environ.get("BASS_TRACE"))

    p = _prep(src, dst, weight, significance, emb, W0, b0, W1, b1)
    n, npc = p["n"], p["npc"]

    nc0 = _build(_conv0_body, tensor_specs0(p), p)
    res0 = run_bass_kernel_spmd(nc0, in_maps0(p), core_ids=list(range(NC_CORES)),
                                trace=trace)
    LAST_RESULTS.append(res0)
    LAST_EXEC_TIMES_NS.append(res0.exec_time_ns)
    hs_full = np.concatenate(
        [np.asarray(res0.results[i]["hs"])[:npc] for i in range(NC_CORES)], axis=0
    )
    assert hs_full.shape == (n, D)

    nc1 = _build(_conv1_body, tensor_specs1(p), p, nq=NQ)
    res1 = run_bass_kernel_spmd(nc1, in_maps1(p, hs_full),
                                core_ids=list(range(NC_CORES)), trace=trace)
    LAST_RESULTS.append(res1)
    LAST_EXEC_TIMES_NS.append(res1.exec_time_ns)

    out = np.concatenate(
        [np.asarray(res1.results[i]["out"])[:npc] for i in range(NC_CORES)], axis=0
    )
    assert out.shape == (n, D)
    return out.astype(np.float32)


# revision 13
# speedup vs baseline: 1.1254x; 1.1059x over previous
"""GCN (2-layer GraphConv, norm='both') on 8 Trainium2 NeuronCores.

Strategy (graph/data parallel, nodes partitioned across cores):
  - Nodes are partitioned into 8 contiguous shards; each core owns its shard's
    in-edges (edges grouped by dst).  Edges are sorted by dst on the host and
    chunked into 128-dst-node groups; per-chunk tile counts are padded to the
    max across the 8 cores so one SPMD program serves all cores.
  - Dispatch 1 (conv0): each core aggregates its in-edges' 4-wide bf16 source
    payloads (host-laid-out, halo-exchange style) via one-hot indicator
    matmuls in PSUM, folds D_in^-1/2 into the PSUM->SBUF copy, applies W0 + b0
    via chained matmuls, and finishes leaky-relu + next-layer D_out^-1/2 in a
    single scalar-engine activation, emitting its hs shard in bf16.
  - Host concatenates the 8 hs shards (pure layout, no math).
  - Dispatch 2 (conv1): each core gathers hs[src] rows (256B bf16) with SWDGE
    dma_gather calls spread over 4 SWDGE queues (descriptor emission
    parallelizes across Q7 core pairs, ~3.7x emission rate), segment-sums per
    128-dst chunk via bf16 one-hot indicator matmuls in PSUM, folds D_in^-1/2
    into the PSUM copy, applies W1 + b1 via chained matmuls, final copy on the
    scalar engine.
  - Host concatenates the 8 output shards.

All O(E*D) / O(N*D) compute and memory traffic runs on-device; the host does
index manipulation (sort/pad/relabel), normalization constants, and the
4-float-per-edge conv0 payload layout.
"""

import os
from contextlib import ExitStack

import ml_dtypes
import numpy as np

import concourse.bass as bass
import concourse.tile as tile
from concourse import bacc, mybir
from concourse._compat import with_exitstack
from concourse.alu_op_type import AluOpType
from concourse.bass_utils import run_bass_kernel_spmd

F32 = mybir.dt.float32
BF16 = mybir.dt.bfloat16
I16 = mybir.dt.int16
BF = ml_dtypes.bfloat16

NC_CORES = 8
D = 128          # feature dim of both conv layers
NEG_SLOPE = 0.01
NQ = 4           # SWDGE queues for gather descriptor emission

# filled by kernel() for test harnesses to inspect
LAST_EXEC_TIMES_NS: list = []
LAST_RESULTS: list = []


# --------------------------------------------------------------------------
# host-side prep
# --------------------------------------------------------------------------

def _wrap_idx(idx: np.ndarray) -> np.ndarray:
    """dma_gather index layout: position i lives at [i % 16, i // 16] of a
    16-row wrap, replicated 8x (one copy per Q7 core) -> [128, n/16] int16."""
    n = idx.shape[0]
    assert n % 16 == 0
    return np.tile(idx.astype(np.int16).reshape(n // 16, 16).T, (8, 1))


def _prep(src, dst, weight, significance, emb, W0, b0, W1, b1):
    n = weight.shape[0]
    npc = n // NC_CORES                    # nodes per core (owned shard)
    assert npc * NC_CORES == n
    n_chunks = (npc + 127) // 128          # 128-dst-node chunks per core
    half = (n + 1) // 2                    # src-id bucket split for int16 idx
    assert half <= 32767 and n - half <= 32767

    src = np.asarray(src).astype(np.int64)
    dst = np.asarray(dst).astype(np.int64)

    out_deg = np.bincount(src, minlength=n).astype(np.float64)
    in_deg = np.bincount(dst, minlength=n).astype(np.float64)
    od = (1.0 / np.sqrt(np.clip(out_deg, 1.0, None))).astype(np.float32)
    ri = (1.0 / np.sqrt(np.clip(in_deg, 1.0, None))).astype(np.float32)

    # conv0 per-edge source payload (halo-exchange layout):
    #   m_e = od[src] * [w[src], emb[sig[src],0], emb[sig[src],1], 0]
    emb_rows = np.asarray(emb, np.float32)[np.asarray(significance).astype(np.int64)]
    feats = np.concatenate(
        [np.asarray(weight, np.float32)[:, None], emb_rows], axis=1
    ) * od[:, None]                                        # [n, 3]

    order = np.argsort(dst, kind="stable")
    s_src, s_dst = src[order], dst[order]

    core_of = s_dst // npc
    loc = s_dst - core_of * npc
    chunk_of = loc // 128
    e_starts = np.searchsorted(core_of * n_chunks + chunk_of,
                               np.arange(NC_CORES * n_chunks + 1))

    es_all = [[None] * n_chunks for _ in range(NC_CORES)]
    dl_all = [[None] * n_chunks for _ in range(NC_CORES)]
    for c in range(NC_CORES):
        for k in range(n_chunks):
            s0, s1 = e_starts[c * n_chunks + k], e_starts[c * n_chunks + k + 1]
            es_all[c][k] = s_src[s0:s1]
            dl_all[c][k] = (s_dst[s0:s1] - c * npc - k * 128).astype(np.float32)

    # conv0 uses 64-dst chunks (halves the DVE one-hot build work)
    nck0 = (npc + 63) // 64
    chunk_of0 = loc // 64
    e_starts0 = np.searchsorted(core_of * nck0 + chunk_of0,
                                np.arange(NC_CORES * nck0 + 1))
    es0_all = [[None] * nck0 for _ in range(NC_CORES)]
    dl0_all = [[None] * nck0 for _ in range(NC_CORES)]
    for c in range(NC_CORES):
        for k in range(nck0):
            s0, s1 = e_starts0[c * nck0 + k], e_starts0[c * nck0 + k + 1]
            es0_all[c][k] = s_src[s0:s1]
            dl0_all[c][k] = (s_dst[s0:s1] - c * npc - k * 64).astype(np.float32)

    # uniform-across-cores tile counts (SPMD: one program for all cores)
    t0 = np.zeros(nck0, np.int64)
    t1 = np.zeros((n_chunks, 2), np.int64)
    for k in range(nck0):
        ne = max(es0_all[c][k].shape[0] for c in range(NC_CORES))
        t0[k] = max(1, -(-ne // 128))
    for k in range(n_chunks):
        for b in range(2):
            nb = max(int(np.count_nonzero((es_all[c][k] < half) == (b == 0)))
                     for c in range(NC_CORES))
            t1[k, b] = -(-nb // 128)
        if t1[k].sum() == 0:
            t1[k, 0] = 1

    T0 = int(t0.sum())
    T1 = int(t1.sum())

    x0h = np.zeros((NC_CORES, 128, T0 * 4), BF)
    dv0 = np.full((NC_CORES, 128, T0), -1.0, BF)
    idxh = np.zeros((NC_CORES, 128, T1 * 8), np.int16)
    dv1 = np.full((NC_CORES, 128, T1), -1.0, BF)

    off0 = np.concatenate([[0], np.cumsum(t0)])
    off1 = np.concatenate([[0], np.cumsum(t1.reshape(-1))]).reshape(-1)

    for c in range(NC_CORES):
        for k in range(nck0):
            es, dloc = es0_all[c][k], dl0_all[c][k]
            ne = es.shape[0]
            # conv0: all edges of the 64-dst chunk, padded to t0[k]*128
            n0 = int(t0[k]) * 128
            pay = np.zeros((n0, 4), np.float32)
            pay[:ne, :3] = feats[es]
            o = int(off0[k])
            x0h[c, :, o * 4:(o + int(t0[k])) * 4] = (
                pay.reshape(int(t0[k]), 128, 4).transpose(1, 0, 2)
                .reshape(128, int(t0[k]) * 4).astype(BF)
            )
            dvc = np.full(n0, -1.0, np.float32)
            dvc[:ne] = dloc
            dv0[c, :, o:o + int(t0[k])] = dvc.reshape(int(t0[k]), 128).T.astype(BF)

        for k in range(n_chunks):
            es, dloc = es_all[c][k], dl_all[c][k]
            # conv1: bucket by src half, pad idx with 0 (gathered, zeroed by
            # the indicator)
            m0 = es < half
            for b, mask in ((0, m0), (1, ~m0)):
                tb = int(t1[k, b])
                if tb == 0:
                    continue
                nb = tb * 128
                sb = es[mask] - (0 if b == 0 else half)
                db = dloc[mask]
                pidx = np.zeros(nb, np.int64)
                pidx[:sb.shape[0]] = sb
                o1 = int(off1[2 * k + b])
                idxh[c, :, o1 * 8:(o1 + tb) * 8] = _wrap_idx(pidx)
                dvb = np.full(nb, -1.0, np.float32)
                dvb[:db.shape[0]] = db
                dv1[c, :, o1:o1 + tb] = dvb.reshape(tb, 128).T.astype(BF)

    # per-core column-replicated 1/sqrt(in_deg): [128, n_chunks*128] f32
    ri_rows = np.ones((NC_CORES, 128, n_chunks * 128), np.float32)
    od_pc = np.ones((NC_CORES, 128, nck0), np.float32)
    for c in range(NC_CORES):
        rv = np.ones(n_chunks * 128, np.float32)
        rv[:npc] = ri[c * npc:(c + 1) * npc]
        ri_rows[c] = rv[None, :]
        ov = np.ones(nck0 * 64, np.float32)
        ov[:npc] = od[c * npc:(c + 1) * npc]
        od_pc[c, :64, :] = ov.reshape(nck0, 64).T

    t0max = int(t0.max())
    ttmax = int(t1.sum(axis=1).max())
    iota0 = np.tile(np.arange(64, dtype=np.float32)[None, :],
                    (128, t0max)).astype(BF)
    iota1 = np.tile(np.arange(128, dtype=np.float32)[None, :],
                    (128, ttmax)).astype(BF)

    consts = {
        "ones_row": np.ones((1, 128), BF),
        "iota0": iota0,
        "iota1": iota1,
        "b0_row": np.asarray(b0, np.float32)[None, :].astype(BF),
        "b1_row": np.asarray(b1, np.float32)[None, :].astype(BF),
        "W0p": np.concatenate(
            [np.asarray(W0, np.float32), np.zeros((1, D), np.float32)], axis=0
        ).astype(BF),
        "W1": np.asarray(W1, np.float32).astype(BF),
    }
    return dict(
        n=n, npc=npc, n_chunks=n_chunks, nck0=nck0, half=half,
        t0=t0, t1=t1, off0=off0, off1=off1, T0=T0, T1=T1,
        t0max=t0max, ttmax=ttmax, t1max=int(t1.max()),
        od_pc=od_pc, ri_rows=ri_rows, x0h=x0h, dv0=dv0, idxh=idxh, dv1=dv1,
        consts=consts,
    )


# --------------------------------------------------------------------------
# device programs
# --------------------------------------------------------------------------

def _new_nc(nq=1):
    return bacc.Bacc("TRN2", target_bir_lowering=False, debug=False,
                     num_devices=NC_CORES, num_swdge_queues=nq)


@with_exitstack
def _conv0_body(ctx: ExitStack, tc, aps, p):
    nc = tc.nc
    nck0, t0, off0, T0 = p["nck0"], p["t0"], p["off0"], p["T0"]
    t0max = p["t0max"]
    cpool = ctx.enter_context(tc.tile_pool(name="consts", bufs=1))
    pool = ctx.enter_context(tc.tile_pool(name="work", bufs=6))
    epool = ctx.enter_context(tc.tile_pool(name="epi", bufs=6))
    ps_a = ctx.enter_context(tc.tile_pool(name="ps_a", bufs=4, space="PSUM"))
    ps_g = ctx.enter_context(tc.tile_pool(name="ps_g", bufs=4, space="PSUM"))

    iota_sb = cpool.tile([128, t0max * 64], BF16)
    nc.sync.dma_start(iota_sb[:], aps["iota0"][:])
    w0_sb = cpool.tile([4, D], BF16)
    nc.sync.dma_start(w0_sb[:], aps["W0p"][:])
    b0r_sb = cpool.tile([1, D], BF16)
    nc.sync.dma_start(b0r_sb[:], aps["b0_row"][:])
    dv0_sb = cpool.tile([128, T0], BF16)
    nc.sync.dma_start(dv0_sb[:], aps["dv0"][:])
    od_sb = cpool.tile([128, nck0], F32)
    nc.sync.dma_start(od_sb[:], aps["od_pc"][:])
    ri_sb = cpool.tile([128, p["n_chunks"] * 128], F32)
    nc.sync.dma_start(ri_sb[:], aps["ri_rows"][:])
    ones1 = cpool.tile([1, 128], BF16)
    nc.sync.dma_start(ones1[:], aps["ones_row"][:])

    x0_sb = cpool.tile([128, T0 * 4], BF16)
    nc.sync.dma_start(x0_sb[:], aps["x0h"][:])
    hs_d = aps["hs"]        # [nck0 * 64, D] bf16 output

    for k in range(nck0):
        tk = int(t0[k])
        o = int(off0[k])
        ind_sb = pool.tile([128, tk * 64], BF16, tag="ind")
        nc.vector.tensor_tensor(
            ind_sb[:].rearrange("p (t j) -> p t j", j=64),
            dv0_sb[:, o:o + tk].unsqueeze(2).broadcast_to([128, tk, 64]),
            iota_sb[:, :tk * 64].rearrange("p (t j) -> p t j", j=64),
            AluOpType.is_equal,
        )
        agg_ps = ps_a.tile([4, 64], F32, tag="agg")
        for t in range(tk):
            nc.tensor.matmul(
                agg_ps[:],
                lhsT=x0_sb[:, (o + t) * 4:(o + t + 1) * 4],
                rhs=ind_sb[:, bass.ts(t, 64)],
                start=(t == 0),
                stop=(t == tk - 1),
            )
        # PSUM->SBUF copy folds the D_in^-1/2 column scale, casts to bf16
        agg_sb = epool.tile([4, 64], BF16, tag="aggsb")
        nc.vector.tensor_tensor(
            agg_sb[:], agg_ps[:], ri_sb[0:4, bass.ts(k, 64)], AluOpType.mult,
        )
        g_ps = ps_g.tile([64, D], F32, tag="g")
        nc.tensor.matmul(g_ps[:], lhsT=agg_sb[:], rhs=w0_sb[:],
                         start=True, stop=False)
        nc.tensor.matmul(g_ps[:], lhsT=ones1[:, :64], rhs=b0r_sb[:],
                         start=False, stop=True)
        # hs = od * leaky_relu(g)  (od > 0 commutes with leaky-relu)
        hs_sb = epool.tile([64, D], BF16, tag="hs")
        nc.scalar.activation(hs_sb[:], g_ps[:], mybir.ActivationFunctionType.Lrelu,
                             scale=od_sb[0:64, k:k + 1], alpha=float(NEG_SLOPE))
        nc.sync.dma_start(hs_d[k * 64:(k + 1) * 64, :], hs_sb[:])


@with_exitstack
def _conv1_body(ctx: ExitStack, tc, aps, p):
    nc = tc.nc
    n_chunks, t1, off1 = p["n_chunks"], p["t1"], p["off1"]
    T1, half, n = p["T1"], p["half"], p["n"]
    ttmax, t1max = p["ttmax"], p["t1max"]
    cpool = ctx.enter_context(tc.tile_pool(name="consts", bufs=1))
    xpool = ctx.enter_context(tc.tile_pool(name="xg", bufs=10))
    ipool = ctx.enter_context(tc.tile_pool(name="ind", bufs=4))
    epool = ctx.enter_context(tc.tile_pool(name="epi", bufs=4))
    ps_a = ctx.enter_context(tc.tile_pool(name="ps_a", bufs=4, space="PSUM"))
    ps_o = ctx.enter_context(tc.tile_pool(name="ps_o", bufs=2, space="PSUM"))

    idx_sb = cpool.tile([128, T1 * 8], I16)
    nc.sync.dma_start(idx_sb[:], aps["idxh"][:])
    iota_sb = cpool.tile([128, ttmax * 128], BF16)
    nc.sync.dma_start(iota_sb[:], aps["iota1"][:])
    w1_sb = cpool.tile([D, D], BF16)
    nc.sync.dma_start(w1_sb[:], aps["W1"][:])
    b1r_sb = cpool.tile([1, D], BF16)
    nc.sync.dma_start(b1r_sb[:], aps["b1_row"][:])
    dv1_sb = cpool.tile([128, T1], BF16)
    nc.sync.dma_start(dv1_sb[:], aps["dv1"][:])
    ri_sb = cpool.tile([128, n_chunks * 128], F32)
    nc.sync.dma_start(ri_sb[:], aps["ri_rows"][:])
    ones1 = cpool.tile([1, 128], BF16)
    nc.sync.dma_start(ones1[:], aps["ones_row"][:])

    hs_d = aps["hs"]        # [n, D] bf16 full pre-scaled features
    out_d = aps["out"]      # [n_chunks * 128, D] f32

    qrr = 0
    for k in range(n_chunks):
        buckets = [(b, int(t1[k, b]), int(off1[2 * k + b]))
                   for b in range(2) if int(t1[k, b]) > 0]
        ttot = sum(tb for _, tb, _ in buckets)

        xgs = []
        for b, tb, o1 in buckets:
            xg = xpool.tile([128, t1max * D], BF16, tag=f"xg{b}")
            src_rows = hs_d[0:half, :] if b == 0 else hs_d[half:n, :]
            nc.gpsimd.dma_gather(
                out_ap=xg[:, :tb * D].rearrange("p (t f) -> p t f", f=D),
                in_ap=src_rows,
                idxs_ap=idx_sb[:, o1 * 8:(o1 + tb) * 8],
                num_idxs=tb * 128,
                num_idxs_reg=tb * 128,
                elem_size=D,
                single_packet=(tb * 8 <= 63),
                queue_num=qrr % NQ,
            )
            qrr += 1
            xgs.append(xg)

        o1_0 = buckets[0][2]
        ind_sb = ipool.tile([128, ttot * 128], BF16, tag="ind")
        nc.vector.tensor_tensor(
            ind_sb[:].rearrange("p (t j) -> p t j", j=128),
            dv1_sb[:, o1_0:o1_0 + ttot].unsqueeze(2).broadcast_to([128, ttot, 128]),
            iota_sb[:, :ttot * 128].rearrange("p (t j) -> p t j", j=128),
            AluOpType.is_equal,
        )
        agg_ps = ps_a.tile([D, 128], F32, tag="agg")
        ti = 0
        for bi, (b, tb, o1) in enumerate(buckets):
            for t in range(tb):
                nc.tensor.matmul(
                    agg_ps[:],
                    lhsT=xgs[bi][:, bass.ts(t, D)],
                    rhs=ind_sb[:, bass.ts(ti, 128)],
                    start=(ti == 0),
                    stop=(ti == ttot - 1),
                )
                ti += 1
        agg_sb = epool.tile([D, 128], BF16, tag="aggsb")
        nc.vector.tensor_tensor(
            agg_sb[:], agg_ps[:], ri_sb[:, bass.ts(k, 128)], AluOpType.mult,
        )
        o_ps = ps_o.tile([128, D], F32, tag="o")
        nc.tensor.matmul(o_ps[:], lhsT=agg_sb[:], rhs=w1_sb[:],
                         start=True, stop=False)
        nc.tensor.matmul(o_ps[:], lhsT=ones1[:], rhs=b1r_sb[:],
                         start=False, stop=True)
        out_sb = epool.tile([128, D], F32, tag="outsb")
        nc.scalar.activation(out_sb[:], o_ps[:],
                             mybir.ActivationFunctionType.Copy)
        nc.sync.dma_start(out_d[k * 128:(k + 1) * 128, :], out_sb[:])


def tensor_specs0(p):
    nck0, T0, t0max = p["nck0"], p["T0"], p["t0max"]
    return {
        "iota0": ((128, t0max * 64), BF16, "ExternalInput"),
        "ones_row": ((1, 128), BF16, "ExternalInput"),
        "b0_row": ((1, D), BF16, "ExternalInput"),
        "W0p": ((4, D), BF16, "ExternalInput"),
        "dv0": ((128, T0), BF16, "ExternalInput"),
        "od_pc": ((128, nck0), F32, "ExternalInput"),
        "ri_rows": ((128, p["n_chunks"] * 128), F32, "ExternalInput"),
        "x0h": ((128, T0 * 4), BF16, "ExternalInput"),
        "hs": ((nck0 * 64, D), BF16, "ExternalOutput"),
    }


def tensor_specs1(p):
    n, n_chunks, T1, ttmax = p["n"], p["n_chunks"], p["T1"], p["ttmax"]
    return {
        "iota1": ((128, ttmax * 128), BF16, "ExternalInput"),
        "ones_row": ((1, 128), BF16, "ExternalInput"),
        "b1_row": ((1, D), BF16, "ExternalInput"),
        "W1": ((D, D), BF16, "ExternalInput"),
        "dv1": ((128, T1), BF16, "ExternalInput"),
        "ri_rows": ((128, n_chunks * 128), F32, "ExternalInput"),
        "idxh": ((128, T1 * 8), I16, "ExternalInput"),
        "hs": ((n, D), BF16, "ExternalInput"),
        "out": ((n_chunks * 128, D), F32, "ExternalOutput"),
    }


def in_maps0(p):
    c = p["consts"]
    return [
        {"iota0": c["iota0"], "ones_row": c["ones_row"],
         "b0_row": c["b0_row"], "W0p": c["W0p"],
         "dv0": p["dv0"][i], "od_pc": p["od_pc"][i], "ri_rows": p["ri_rows"][i],
         "x0h": p["x0h"][i]}
        for i in range(NC_CORES)
    ]


def in_maps1(p, hs_full):
    c = p["consts"]
    return [
        {"iota1": c["iota1"], "ones_row": c["ones_row"],
         "b1_row": c["b1_row"], "W1": c["W1"],
         "dv1": p["dv1"][i], "ri_rows": p["ri_rows"][i], "idxh": p["idxh"][i],
         "hs": hs_full}
        for i in range(NC_CORES)
    ]


def _build(body, tensors, p, nq=1):
    nc = _new_nc(nq)
    aps = {
        name: nc.dram_tensor(name, list(shape), dtype, kind=kind).ap()
        for name, (shape, dtype, kind) in tensors.items()
    }
    with tile.TileContext(nc) as tc:
        body(tc, aps, p)
    nc.compile()
    return nc


# --------------------------------------------------------------------------
# entry point
# --------------------------------------------------------------------------

def kernel(src, dst, weight, significance, emb, W0, b0, W1, b1):
    global LAST_EXEC_TIMES_NS, LAST_RESULTS
    LAST_EXEC_TIMES_NS = []
    LAST_RESULTS = []
    trace = bool(os.environ.get("BASS_TRACE"))

    p = _prep(src, dst, weight, significance, emb, W0, b0, W1, b1)
    n, npc = p["n"], p["npc"]

    nc0 = _build(_conv0_body, tensor_specs0(p), p)
    res0 = run_bass_kernel_spmd(nc0, in_maps0(p), core_ids=list(range(NC_CORES)),
                                trace=trace)
    LAST_RESULTS.append(res0)
    LAST_EXEC_TIMES_NS.append(res0.exec_time_ns)
    hs_full = np.concatenate(
        [np.asarray(res0.results[i]["hs"])[:npc] for i in range(NC_CORES)], axis=0
    )
    assert hs_full.shape == (n, D)

    nc1 = _build(_conv1_body, tensor_specs1(p), p, nq=NQ)
    res1 = run_bass_kernel_spmd(nc1, in_maps1(p, hs_full),
                                core_ids=list(range(NC_CORES)), trace=trace)
    LAST_RESULTS.append(res1)
    LAST_EXEC_TIMES_NS.append(res1.exec_time_ns)

    out = np.concatenate(
        [np.asarray(res1.results[i]["out"])[:npc] for i in range(NC_CORES)], axis=0
    )
    assert out.shape == (n, D)
    return out.astype(np.float32)


# revision 14
# speedup vs baseline: 1.2095x; 1.0747x over previous
"""GCN (2-layer GraphConv, norm='both') on 8 Trainium2 NeuronCores.

Strategy (graph/data parallel, nodes partitioned across cores):
  - Nodes are partitioned into 8 contiguous shards; each core owns its shard's
    in-edges (edges grouped by dst).  Edges are sorted by dst on the host and
    chunked into 128-dst-node groups; per-chunk tile counts are padded to the
    max across the 8 cores so one SPMD program serves all cores.
  - Dispatch 1 (conv0): each core aggregates its in-edges' 4-wide bf16 source
    payloads (host-laid-out, halo-exchange style) via one-hot indicator
    matmuls in PSUM, folds D_in^-1/2 into the PSUM->SBUF copy, applies W0 + b0
    via chained matmuls, and finishes leaky-relu + next-layer D_out^-1/2 in a
    single scalar-engine activation, emitting its hs shard in bf16.
  - Host concatenates the 8 hs shards (pure layout, no math).
  - Dispatch 2 (conv1): each core gathers hs[src] rows (256B bf16) with SWDGE
    dma_gather calls spread over 4 SWDGE queues (descriptor emission
    parallelizes across Q7 core pairs, ~3.7x emission rate), segment-sums per
    128-dst chunk via bf16 one-hot indicator matmuls in PSUM, folds D_in^-1/2
    into the PSUM copy, applies W1 + b1 via chained matmuls, final copy on the
    scalar engine.
  - Host concatenates the 8 output shards.

All O(E*D) / O(N*D) compute and memory traffic runs on-device; the host does
index manipulation (sort/pad/relabel), normalization constants, and the
4-float-per-edge conv0 payload layout.
"""

import os
from contextlib import ExitStack

import ml_dtypes
import numpy as np

import concourse.bass as bass
import concourse.tile as tile
from concourse import bacc, mybir
from concourse._compat import with_exitstack
from concourse.alu_op_type import AluOpType
from concourse.bass_utils import run_bass_kernel_spmd

F32 = mybir.dt.float32
BF16 = mybir.dt.bfloat16
I16 = mybir.dt.int16
BF = ml_dtypes.bfloat16

NC_CORES = 8
D = 128          # feature dim of both conv layers
NEG_SLOPE = 0.01
NQ = 4           # SWDGE queues for gather descriptor emission

# filled by kernel() for test harnesses to inspect
LAST_EXEC_TIMES_NS: list = []
LAST_RESULTS: list = []


# --------------------------------------------------------------------------
# host-side prep
# --------------------------------------------------------------------------

def _wrap_idx(idx: np.ndarray) -> np.ndarray:
    """dma_gather index layout: position i lives at [i % 16, i // 16] of a
    16-row wrap, replicated 8x (one copy per Q7 core) -> [128, n/16] int16."""
    n = idx.shape[0]
    assert n % 16 == 0
    return np.tile(idx.astype(np.int16).reshape(n // 16, 16).T, (8, 1))


def _prep(src, dst, weight, significance, emb, W0, b0, W1, b1):
    n = weight.shape[0]
    npc = n // NC_CORES                    # nodes per core (owned shard)
    assert npc * NC_CORES == n
    n_chunks = (npc + 127) // 128          # 128-dst-node chunks per core
    half = (n + 1) // 2                    # src-id bucket split for int16 idx
    assert half <= 32767 and n - half <= 32767

    src = np.asarray(src).astype(np.int64)
    dst = np.asarray(dst).astype(np.int64)

    out_deg = np.bincount(src, minlength=n).astype(np.float64)
    in_deg = np.bincount(dst, minlength=n).astype(np.float64)
    od = (1.0 / np.sqrt(np.clip(out_deg, 1.0, None))).astype(np.float32)
    ri = (1.0 / np.sqrt(np.clip(in_deg, 1.0, None))).astype(np.float32)

    # conv0 per-edge source payload (halo-exchange layout):
    #   m_e = od[src] * [w[src], emb[sig[src],0], emb[sig[src],1], 0]
    emb_rows = np.asarray(emb, np.float32)[np.asarray(significance).astype(np.int64)]
    feats = np.concatenate(
        [np.asarray(weight, np.float32)[:, None], emb_rows], axis=1
    ) * od[:, None]                                        # [n, 3]

    order = np.argsort(dst, kind="stable")
    s_src, s_dst = src[order], dst[order]

    core_of = s_dst // npc
    loc = s_dst - core_of * npc
    chunk_of = loc // 128
    e_starts = np.searchsorted(core_of * n_chunks + chunk_of,
                               np.arange(NC_CORES * n_chunks + 1))

    es_all = [[None] * n_chunks for _ in range(NC_CORES)]
    dl_all = [[None] * n_chunks for _ in range(NC_CORES)]
    for c in range(NC_CORES):
        for k in range(n_chunks):
            s0, s1 = e_starts[c * n_chunks + k], e_starts[c * n_chunks + k + 1]
            es_all[c][k] = s_src[s0:s1]
            dl_all[c][k] = (s_dst[s0:s1] - c * npc - k * 128).astype(np.float32)

    # conv0 uses 64-dst chunks (halves the DVE one-hot build work)
    nck0 = (npc + 63) // 64
    chunk_of0 = loc // 64
    e_starts0 = np.searchsorted(core_of * nck0 + chunk_of0,
                                np.arange(NC_CORES * nck0 + 1))
    es0_all = [[None] * nck0 for _ in range(NC_CORES)]
    dl0_all = [[None] * nck0 for _ in range(NC_CORES)]
    for c in range(NC_CORES):
        for k in range(nck0):
            s0, s1 = e_starts0[c * nck0 + k], e_starts0[c * nck0 + k + 1]
            es0_all[c][k] = s_src[s0:s1]
            dl0_all[c][k] = (s_dst[s0:s1] - c * npc - k * 64).astype(np.float32)

    # uniform-across-cores tile counts (SPMD: one program for all cores)
    t0 = np.zeros(nck0, np.int64)
    t1 = np.zeros((n_chunks, 2), np.int64)
    for k in range(nck0):
        ne = max(es0_all[c][k].shape[0] for c in range(NC_CORES))
        t0[k] = max(1, -(-ne // 128))
    for k in range(n_chunks):
        for b in range(2):
            nb = max(int(np.count_nonzero((es_all[c][k] < half) == (b == 0)))
                     for c in range(NC_CORES))
            t1[k, b] = -(-nb // 128)
        if t1[k].sum() == 0:
            t1[k, 0] = 1

    T0 = int(t0.sum())
    T1 = int(t1.sum())

    x0h = np.zeros((NC_CORES, 128, T0 * 4), BF)
    dv0 = np.full((NC_CORES, 128, T0), -1.0, BF)
    idxh = np.zeros((NC_CORES, 128, T1 * 8), np.int16)
    dv1 = np.full((NC_CORES, 128, T1), -1.0, BF)

    off0 = np.concatenate([[0], np.cumsum(t0)])
    off1 = np.concatenate([[0], np.cumsum(t1.reshape(-1))]).reshape(-1)

    for c in range(NC_CORES):
        for k in range(nck0):
            es, dloc = es0_all[c][k], dl0_all[c][k]
            ne = es.shape[0]
            # conv0: all edges of the 64-dst chunk, padded to t0[k]*128
            n0 = int(t0[k]) * 128
            pay = np.zeros((n0, 4), np.float32)
            pay[:ne, :3] = feats[es]
            o = int(off0[k])
            x0h[c, :, o * 4:(o + int(t0[k])) * 4] = (
                pay.reshape(int(t0[k]), 128, 4).transpose(1, 0, 2)
                .reshape(128, int(t0[k]) * 4).astype(BF)
            )
            dvc = np.full(n0, -1.0, np.float32)
            dvc[:ne] = dloc
            dv0[c, :, o:o + int(t0[k])] = dvc.reshape(int(t0[k]), 128).T.astype(BF)

        for k in range(n_chunks):
            es, dloc = es_all[c][k], dl_all[c][k]
            # conv1: bucket by src half, pad idx with 0 (gathered, zeroed by
            # the indicator)
            m0 = es < half
            for b, mask in ((0, m0), (1, ~m0)):
                tb = int(t1[k, b])
                if tb == 0:
                    continue
                nb = tb * 128
                sb = es[mask] - (0 if b == 0 else half)
                db = dloc[mask]
                pidx = np.zeros(nb, np.int64)
                pidx[:sb.shape[0]] = sb
                o1 = int(off1[2 * k + b])
                idxh[c, :, o1 * 8:(o1 + tb) * 8] = _wrap_idx(pidx)
                dvb = np.full(nb, -1.0, np.float32)
                dvb[:db.shape[0]] = db
                dv1[c, :, o1:o1 + tb] = dvb.reshape(tb, 128).T.astype(BF)

    # per-core column-replicated 1/sqrt(in_deg): [128, n_chunks*128] f32
    ri_rows = np.ones((NC_CORES, 128, n_chunks * 128), np.float32)
    od_pc = np.ones((NC_CORES, 128, nck0), np.float32)
    for c in range(NC_CORES):
        rv = np.ones(n_chunks * 128, np.float32)
        rv[:npc] = ri[c * npc:(c + 1) * npc]
        ri_rows[c] = rv[None, :]
        ov = np.ones(nck0 * 64, np.float32)
        ov[:npc] = od[c * npc:(c + 1) * npc]
        od_pc[c, :64, :] = ov.reshape(nck0, 64).T

    t0max = int(t0.max())
    ttmax = int(t1.sum(axis=1).max())
    iota0 = np.tile(np.arange(64, dtype=np.float32)[None, :],
                    (128, t0max)).astype(BF)
    iota1 = np.tile(np.arange(128, dtype=np.float32)[None, :],
                    (128, ttmax)).astype(BF)

    consts = {
        "ones_row": np.ones((1, 128), BF),
        "iota0": iota0,
        "iota1": iota1,
        "b0_row": np.asarray(b0, np.float32)[None, :].astype(BF),
        "b1_row": np.asarray(b1, np.float32)[None, :].astype(BF),
        "W0p": np.concatenate(
            [np.asarray(W0, np.float32), np.zeros((1, D), np.float32)], axis=0
        ).astype(BF),
        "W1": np.asarray(W1, np.float32).astype(BF),
    }
    return dict(
        n=n, npc=npc, n_chunks=n_chunks, nck0=nck0, half=half,
        t0=t0, t1=t1, off0=off0, off1=off1, T0=T0, T1=T1,
        t0max=t0max, ttmax=ttmax, t1max=int(t1.max()),
        od_pc=od_pc, ri_rows=ri_rows, x0h=x0h, dv0=dv0, idxh=idxh, dv1=dv1,
        consts=consts,
    )


# --------------------------------------------------------------------------
# device programs
# --------------------------------------------------------------------------

def _new_nc(nq=1):
    return bacc.Bacc("TRN2", target_bir_lowering=False, debug=False,
                     num_devices=NC_CORES, num_swdge_queues=nq)


@with_exitstack
def _conv0_body(ctx: ExitStack, tc, aps, p):
    nc = tc.nc
    nck0, t0, off0, T0 = p["nck0"], p["t0"], p["off0"], p["T0"]
    t0max = p["t0max"]
    cpool = ctx.enter_context(tc.tile_pool(name="consts", bufs=1))
    pool = ctx.enter_context(tc.tile_pool(name="work", bufs=6))
    epool = ctx.enter_context(tc.tile_pool(name="epi", bufs=6))
    ps_a = ctx.enter_context(tc.tile_pool(name="ps_a", bufs=4, space="PSUM"))
    ps_g = ctx.enter_context(tc.tile_pool(name="ps_g", bufs=4, space="PSUM"))

    iota_sb = cpool.tile([128, t0max * 64], BF16)
    nc.sync.dma_start(iota_sb[:], aps["iota0"][:])
    w0_sb = cpool.tile([4, D], BF16)
    nc.sync.dma_start(w0_sb[:], aps["W0p"][:])
    b0r_sb = cpool.tile([1, D], BF16)
    nc.sync.dma_start(b0r_sb[:], aps["b0_row"][:])
    dv0_sb = cpool.tile([128, T0], BF16)
    nc.sync.dma_start(dv0_sb[:], aps["dv0"][:])
    od_sb = cpool.tile([128, nck0], F32)
    nc.sync.dma_start(od_sb[:], aps["od_pc"][:])
    ri_sb = cpool.tile([128, p["n_chunks"] * 128], F32)
    nc.sync.dma_start(ri_sb[:], aps["ri_rows"][:])
    ones1 = cpool.tile([1, 128], BF16)
    nc.sync.dma_start(ones1[:], aps["ones_row"][:])

    x0_sb = cpool.tile([128, T0 * 4], BF16)
    nc.sync.dma_start(x0_sb[:], aps["x0h"][:])
    hs_d = aps["hs"]        # [nck0 * 64, D] bf16 output

    for k in range(nck0):
        tk = int(t0[k])
        o = int(off0[k])
        ind_sb = pool.tile([128, tk * 64], BF16, tag="ind")
        nc.vector.tensor_tensor(
            ind_sb[:].rearrange("p (t j) -> p t j", j=64),
            dv0_sb[:, o:o + tk].unsqueeze(2).broadcast_to([128, tk, 64]),
            iota_sb[:, :tk * 64].rearrange("p (t j) -> p t j", j=64),
            AluOpType.is_equal,
        )
        agg_ps = ps_a.tile([4, 64], F32, tag="agg")
        for t in range(tk):
            nc.tensor.matmul(
                agg_ps[:],
                lhsT=x0_sb[:, (o + t) * 4:(o + t + 1) * 4],
                rhs=ind_sb[:, bass.ts(t, 64)],
                start=(t == 0),
                stop=(t == tk - 1),
            )
        # PSUM->SBUF copy folds the D_in^-1/2 column scale, casts to bf16
        agg_sb = epool.tile([4, 64], BF16, tag="aggsb")
        nc.vector.tensor_tensor(
            agg_sb[:], agg_ps[:], ri_sb[0:4, bass.ts(k, 64)], AluOpType.mult,
        )
        g_ps = ps_g.tile([64, D], F32, tag="g")
        nc.tensor.matmul(g_ps[:], lhsT=agg_sb[:], rhs=w0_sb[:],
                         start=True, stop=False)
        nc.tensor.matmul(g_ps[:], lhsT=ones1[:, :64], rhs=b0r_sb[:],
                         start=False, stop=True)
        # hs = od * leaky_relu(g)  (od > 0 commutes with leaky-relu)
        hs_sb = epool.tile([64, D], BF16, tag="hs")
        nc.scalar.activation(hs_sb[:], g_ps[:], mybir.ActivationFunctionType.Lrelu,
                             scale=od_sb[0:64, k:k + 1], alpha=float(NEG_SLOPE))
        nc.sync.dma_start(hs_d[k * 64:(k + 1) * 64, :], hs_sb[:])


@with_exitstack
def _conv1_body(ctx: ExitStack, tc, aps, p):
    nc = tc.nc
    n_chunks, t1, off1 = p["n_chunks"], p["t1"], p["off1"]
    T1, half, n = p["T1"], p["half"], p["n"]
    ttmax, t1max = p["ttmax"], p["t1max"]
    cpool = ctx.enter_context(tc.tile_pool(name="consts", bufs=1))
    xpool = ctx.enter_context(tc.tile_pool(name="xg", bufs=10))
    ipool = ctx.enter_context(tc.tile_pool(name="ind", bufs=6))
    epool = ctx.enter_context(tc.tile_pool(name="epi", bufs=4))
    ps_a = ctx.enter_context(tc.tile_pool(name="ps_a", bufs=4, space="PSUM"))
    ps_o = ctx.enter_context(tc.tile_pool(name="ps_o", bufs=2, space="PSUM"))

    idx_sb = cpool.tile([128, T1 * 8], I16)
    cut = min(int(off1[8]) * 8, T1 * 8)   # first 4 chunks' idx land first
    nc.sync.dma_start(idx_sb[:, :cut], aps["idxh"][:, :cut])
    nc.sync.dma_start(idx_sb[:, cut:], aps["idxh"][:, cut:])
    iota_sb = cpool.tile([128, ttmax * 128], BF16)
    nc.sync.dma_start(iota_sb[:], aps["iota1"][:])
    w1_sb = cpool.tile([D, D], BF16)
    nc.sync.dma_start(w1_sb[:], aps["W1"][:])
    b1r_sb = cpool.tile([1, D], BF16)
    nc.sync.dma_start(b1r_sb[:], aps["b1_row"][:])
    dv1_sb = cpool.tile([128, T1], BF16)
    nc.sync.dma_start(dv1_sb[:], aps["dv1"][:])
    ri_sb = cpool.tile([128, n_chunks * 128], F32)
    nc.sync.dma_start(ri_sb[:], aps["ri_rows"][:])
    ones1 = cpool.tile([1, 128], BF16)
    nc.sync.dma_start(ones1[:], aps["ones_row"][:])

    hs_d = aps["hs"]        # [n, D] bf16 full pre-scaled features
    out_d = aps["out"]      # [n_chunks * 128, D] f32

    qload = [0] * NQ
    for k in range(n_chunks):
        buckets = [(b, int(t1[k, b]), int(off1[2 * k + b]))
                   for b in range(2) if int(t1[k, b]) > 0]
        ttot = sum(tb for _, tb, _ in buckets)

        xgs = []
        for b, tb, o1 in buckets:
            qn = min(range(NQ), key=lambda q: qload[q])
            xg = xpool.tile([128, t1max * D], BF16, tag=f"xg{b}")
            src_rows = hs_d[0:half, :] if b == 0 else hs_d[half:n, :]
            nc.gpsimd.dma_gather(
                out_ap=xg[:, :tb * D].rearrange("p (t f) -> p t f", f=D),
                in_ap=src_rows,
                idxs_ap=idx_sb[:, o1 * 8:(o1 + tb) * 8],
                num_idxs=tb * 128,
                num_idxs_reg=tb * 128,
                elem_size=D,
                single_packet=(tb * 8 <= 63),
                queue_num=qn,
            )
            qload[qn] += tb
            xgs.append(xg)

        o1_0 = buckets[0][2]
        ind_sb = ipool.tile([128, ttot * 128], BF16, tag="ind")
        nc.vector.tensor_tensor(
            ind_sb[:].rearrange("p (t j) -> p t j", j=128),
            dv1_sb[:, o1_0:o1_0 + ttot].unsqueeze(2).broadcast_to([128, ttot, 128]),
            iota_sb[:, :ttot * 128].rearrange("p (t j) -> p t j", j=128),
            AluOpType.is_equal,
        )
        agg_ps = ps_a.tile([D, 128], F32, tag="agg")
        ti = 0
        for bi, (b, tb, o1) in enumerate(buckets):
            for t in range(tb):
                nc.tensor.matmul(
                    agg_ps[:],
                    lhsT=xgs[bi][:, bass.ts(t, D)],
                    rhs=ind_sb[:, bass.ts(ti, 128)],
                    start=(ti == 0),
                    stop=(ti == ttot - 1),
                )
                ti += 1
        agg_sb = epool.tile([D, 128], BF16, tag="aggsb")
        nc.vector.tensor_tensor(
            agg_sb[:], agg_ps[:], ri_sb[:, bass.ts(k, 128)], AluOpType.mult,
        )
        o_ps = ps_o.tile([128, D], F32, tag="o")
        nc.tensor.matmul(o_ps[:], lhsT=agg_sb[:], rhs=w1_sb[:],
                         start=True, stop=False)
        nc.tensor.matmul(o_ps[:], lhsT=ones1[:], rhs=b1r_sb[:],
                         start=False, stop=True)
        out_sb = epool.tile([128, D], F32, tag="outsb")
        nc.scalar.activation(out_sb[:], o_ps[:],
                             mybir.ActivationFunctionType.Copy)
        nc.sync.dma_start(out_d[k * 128:(k + 1) * 128, :], out_sb[:])


def tensor_specs0(p):
    nck0, T0, t0max = p["nck0"], p["T0"], p["t0max"]
    return {
        "iota0": ((128, t0max * 64), BF16, "ExternalInput"),
        "ones_row": ((1, 128), BF16, "ExternalInput"),
        "b0_row": ((1, D), BF16, "ExternalInput"),
        "W0p": ((4, D), BF16, "ExternalInput"),
        "dv0": ((128, T0), BF16, "ExternalInput"),
        "od_pc": ((128, nck0), F32, "ExternalInput"),
        "ri_rows": ((128, p["n_chunks"] * 128), F32, "ExternalInput"),
        "x0h": ((128, T0 * 4), BF16, "ExternalInput"),
        "hs": ((nck0 * 64, D), BF16, "ExternalOutput"),
    }


def tensor_specs1(p):
    n, n_chunks, T1, ttmax = p["n"], p["n_chunks"], p["T1"], p["ttmax"]
    return {
        "iota1": ((128, ttmax * 128), BF16, "ExternalInput"),
        "ones_row": ((1, 128), BF16, "ExternalInput"),
        "b1_row": ((1, D), BF16, "ExternalInput"),
        "W1": ((D, D), BF16, "ExternalInput"),
        "dv1": ((128, T1), BF16, "ExternalInput"),
        "ri_rows": ((128, n_chunks * 128), F32, "ExternalInput"),
        "idxh": ((128, T1 * 8), I16, "ExternalInput"),
        "hs": ((n, D), BF16, "ExternalInput"),
        "out": ((n_chunks * 128, D), F32, "ExternalOutput"),
    }


def in_maps0(p):
    c = p["consts"]
    return [
        {"iota0": c["iota0"], "ones_row": c["ones_row"],
         "b0_row": c["b0_row"], "W0p": c["W0p"],
         "dv0": p["dv0"][i], "od_pc": p["od_pc"][i], "ri_rows": p["ri_rows"][i],
         "x0h": p["x0h"][i]}
        for i in range(NC_CORES)
    ]


def in_maps1(p, hs_full):
    c = p["consts"]
    return [
        {"iota1": c["iota1"], "ones_row": c["ones_row"],
         "b1_row": c["b1_row"], "W1": c["W1"],
         "dv1": p["dv1"][i], "ri_rows": p["ri_rows"][i], "idxh": p["idxh"][i],
         "hs": hs_full}
        for i in range(NC_CORES)
    ]


def _build(body, tensors, p, nq=1):
    nc = _new_nc(nq)
    aps = {
        name: nc.dram_tensor(name, list(shape), dtype, kind=kind).ap()
        for name, (shape, dtype, kind) in tensors.items()
    }
    with tile.TileContext(nc) as tc:
        body(tc, aps, p)
    nc.compile()
    return nc


# --------------------------------------------------------------------------
# entry point
# --------------------------------------------------------------------------

def kernel(src, dst, weight, significance, emb, W0, b0, W1, b1):
    global LAST_EXEC_TIMES_NS, LAST_RESULTS
    LAST_EXEC_TIMES_NS = []
    LAST_RESULTS = []
    trace = bool(os.environ.get("BASS_TRACE"))

    p = _prep(src, dst, weight, significance, emb, W0, b0, W1, b1)
    n, npc = p["n"], p["npc"]

    nc0 = _build(_conv0_body, tensor_specs0(p), p)
    res0 = run_bass_kernel_spmd(nc0, in_maps0(p), core_ids=list(range(NC_CORES)),
                                trace=trace)
    LAST_RESULTS.append(res0)
    LAST_EXEC_TIMES_NS.append(res0.exec_time_ns)
    hs_full = np.concatenate(
        [np.asarray(res0.results[i]["hs"])[:npc] for i in range(NC_CORES)], axis=0
    )
    assert hs_full.shape == (n, D)

    nc1 = _build(_conv1_body, tensor_specs1(p), p, nq=NQ)
    res1 = run_bass_kernel_spmd(nc1, in_maps1(p, hs_full),
                                core_ids=list(range(NC_CORES)), trace=trace)
    LAST_RESULTS.append(res1)
    LAST_EXEC_TIMES_NS.append(res1.exec_time_ns)

    out = np.concatenate(
        [np.asarray(res1.results[i]["out"])[:npc] for i in range(NC_CORES)], axis=0
    )
    assert out.shape == (n, D)
    return out.astype(np.float32)
